# revision 6
# baseline (speedup 1.0000x reference)
"""Multi-head (per-task) 2-layer MLP classifier for Trainium2, 8 NeuronCores.

Strategy: expert-parallel with host-side dispatch. Rows of x are grouped by
task_id on the host (the all-to-all "dispatch"); core t gets all rows whose
task_id == t, zero-padded to a fixed PAD columns, pre-transposed to x^T
[D, PAD]. Each core runs a dense 2-layer MLP for its own head only:

    H^T = relu(W1^T x^T + b1)        [H, PAD]   (psum: out=W1.T@xT, lhsT=W1)
    Y^T = W2^T H^T + b2              [C, PAD]   (lhsT=W2, rhs=H^T)

Everything stays "transposed" (feature dim on partitions, batch on the free
dim) so both matmuls chain without any on-device transpose, and both biases
are per-partition vectors. The host scatters Y^T columns back to the
original row order.

Schedule (v2): the batch columns split into phase A (all full 512-col chunks
except the last) and phase B (last full chunk + the % 512 tail).

  Phase A is k-outer so the PE consumes W1 k-tiles as they stream from HBM
  (W1 split across the scalar+vector DGE rings, x^T on the sync ring). The
  m-sweep is split in halves of 4 PSUM banks so a 6-buffer PSUM ring covers
  A's accumulators, layer-2, and all of phase B's rotating tiles, leaving 2
  banks for B's persistent layer-2 accumulators.

  Phase B is m-outer with the tail chunk folded into the same stationary
  weight load: LDW(w1[k,m]) then matmul both the 512-wide and the tail
  columns. This removes the per-(k,m) LDWEIGHTS cost of a separate tail
  chunk (the tail is LDW-bandwidth-bound: N=16 streams in 16 cycles but a
  weight load is ~128). Layer-2 accumulates into persistent PSUM across m
  and is spliced into the middle of the next m's k-sweep so the PE never
  waits on the relu.

  Relus (bias+max fused) alternate between the DVE (vector) and the
  otherwise-idle Activation (scalar) engine to halve the burst at each
  k-sweep boundary. A few warmup matmuls on memset data run during the
  initial DMA fill so the PE_HAM clock gate reaches 2.4 GHz by the time
  real matmuls start.
"""

import os

import numpy as np

import concourse.bacc as bacc
import concourse.bass as bass
import concourse.mybir as mybir
import concourse.tile as tile
from concourse.bass_utils import run_bass_kernel_spmd

# Problem constants (nn_MultiHeadClassifier: T tasks, 2-layer MLP heads)
T = 8          # tasks == cores
D = 1024       # d_model
HID = 1024     # hidden
C = 100        # classes
B = 8192       # batch
P = 128        # partitions
KD = D // P    # k-tiles in layer-1 contraction
KH = HID // P  # k-tiles in layer-2 contraction

# Per-core padded batch. Task counts for the graded inputs max out at 1040;
# _run grows this automatically if a different distribution needs more.
PAD_DEFAULT = 1040


def build_program(pad, n_warm=7):
    """One SPMD NeuronCore program: dense 2-layer MLP on [D, pad] x^T."""
    bf16 = mybir.dt.bfloat16
    f32 = mybir.dt.float32
    relu = mybir.ActivationFunctionType.Relu
    add = mybir.AluOpType.add
    mx = mybir.AluOpType.max

    n_full = pad // 512
    tail = pad % 512
    # Phase A: full chunks 0..n_full-2 (k-outer, streaming).
    # Phase B: last full chunk + tail, m-outer with shared stationaries.
    a_chunks = [(i * 512, 512) for i in range(max(n_full - 1, 0))]
    if n_full:
        b_main = ((n_full - 1) * 512, 512)
    else:
        b_main = (0, 0)
    b_tail = (n_full * 512, tail) if tail else None

    nc = bacc.Bacc()
    xt = nc.dram_tensor("xt", [D, pad], bf16, kind="ExternalInput")
    w1 = nc.dram_tensor("w1", [D, HID], bf16, kind="ExternalInput")
    b1 = nc.dram_tensor("b1", [P, KH], f32, kind="ExternalInput")
    w2 = nc.dram_tensor("w2", [HID, P], bf16, kind="ExternalInput")
    b2 = nc.dram_tensor("b2", [C, 1], f32, kind="ExternalInput")
    yt = nc.dram_tensor("yt", [C, pad], f32, kind="ExternalOutput")

    w1_t = w1.rearrange("(k p) h -> k p h", p=P)
    xt_t = xt.rearrange("(k p) b -> k p b", p=P)

    with tile.TileContext(nc) as tc:
        with (
            tc.tile_pool(name="weights", bufs=1) as wpool,
            tc.tile_pool(name="acts", bufs=1) as apool,
            tc.tile_pool(name="ps", bufs=7, space="PSUM") as pspool,
            tc.tile_pool(name="ps2", bufs=1, space="PSUM") as ps2pool,
            tc.tile_pool(name="outs", bufs=3) as opool,
        ):
            # ---- DMA plan ----------------------------------------------
            # Two HWDGE rings (scalar, sync) + gpsimd SWDGE. w1 interleaves
            # across both rings (evens on scalar, odds on sync) so phase A's
            # k-sweep is fed at ~2x one ring's rate; k0 is split in halves
            # so the very first LDWEIGHTS waits on 128KB, not 256KB. x^T
            # phase-A pieces ride the sync ring between w1 odds; the phase-B
            # pieces go to whichever ring frees up first. Biases + w2 on the
            # SWDGE (slow but off the critical rings).
            w1_half = {}  # k=0 halves
            w1_sb = [None] * KD
            w1_half[0] = wpool.tile([P, 512], bf16, name="w1_0a", tag="w1_0a")
            nc.scalar.dma_start(out=w1_half[0][:], in_=w1_t[0, :, 0:512])
            w1_half[1] = wpool.tile([P, 512], bf16, name="w1_0b", tag="w1_0b")
            nc.scalar.dma_start(out=w1_half[1][:], in_=w1_t[0, :, 512:1024])

            def w1_ap(k, m):
                """lhsT [128, 128] for layer-1 block (k, m)."""
                if k == 0:
                    h = w1_half[m // 4]
                    return h[:, (m % 4) * P:(m % 4 + 1) * P]
                return w1_sb[k][:, m * P:(m + 1) * P]

            all_chunks = a_chunks + [b_main] + ([b_tail] if b_tail else [])
            xt_sb = {}

            def xt_dma(eng, k, ci):
                o, cw = all_chunks[ci]
                t = wpool.tile([P, cw], bf16, name=f"xt_{k}_{ci}",
                               tag=f"xt_{k}_{ci}")
                eng.dma_start(out=t[:], in_=xt_t[k, :, o:o + cw])
                xt_sb[(k, ci)] = t

            # sync: first phase-A x^T piece, then w1 odds interleaved with
            # the remaining phase-A x^T pieces (arrival tracks consumption)
            xt_dma(nc.sync, 0, 0)
            for k in range(1, KD):
                if k % 2:
                    w1_sb[k] = wpool.tile([P, HID], bf16, name=f"w1_{k}",
                                          tag=f"w1_{k}")
                    nc.sync.dma_start(out=w1_sb[k][:], in_=w1_t[k, :, :])
                else:
                    w1_sb[k] = wpool.tile([P, HID], bf16, name=f"w1_{k}",
                                          tag=f"w1_{k}")
                    nc.scalar.dma_start(out=w1_sb[k][:], in_=w1_t[k, :, :])
                xt_dma(nc.sync, k, 0)
            # remaining phase-A chunks (none for pad=1040), then phase B:
            # first half of the B-main pieces on sync, second half + tail on
            # scalar (which is idle after the w1 evens)
            for ci in range(1, len(a_chunks)):
                for k in range(KD):
                    xt_dma(nc.sync, k, ci)
            ci_bm = len(a_chunks)
            for k in range(KD):
                xt_dma(nc.sync if k < KD // 2 else nc.scalar, k, ci_bm)
            if b_tail:
                for k in range(KD):
                    xt_dma(nc.scalar, k, ci_bm + 1)

            b1_sb = wpool.tile([P, KH], f32, name="b1", tag="b1")
            nc.gpsimd.dma_start(out=b1_sb[:], in_=b1[:])
            b2_sb = wpool.tile([C, 1], f32, name="b2", tag="b2")
            nc.gpsimd.dma_start(out=b2_sb[:], in_=b2[:])

            w2_all = wpool.tile([P, KH, P], bf16, name="w2_all", tag="w2_all")
            nc.gpsimd.dma_start(
                out=w2_all[:],
                in_=w2.rearrange("(k p) c -> p k c", p=P),
            )
            w2_sb = [w2_all[:, k, :] for k in range(KH)]

            # ---- PE warmup ---------------------------------------------
            # The PE queue clears the framework preamble ~3us before the
            # first W1 piece lands. Matmuls on memset data keep the PE_HAM
            # activity window busy so the clock un-gates to 2.4 GHz around
            # when real matmuls begin (and they cost nothing: the PE would
            # otherwise idle).
            if n_warm:
                warm = wpool.tile([P, 512], bf16, name="warm", tag="warm")
                nc.vector.memset(warm[:], 0.0)
                pw = pspool.tile([P, 512], f32, name="ps_w", tag="psring")
                for w in range(n_warm):
                    nc.tensor.matmul(
                        out=pw[:], lhsT=warm[:, 0:P], rhs=warm[:],
                        start=(w == 0), stop=(w == n_warm - 1),
                    )

            h_sb = [apool.tile([P, pad], bf16, name=f"h_{m}", tag=f"h_{m}")
                    for m in range(KH)]

            def relu_bias(m, ps_ap, o, cw):
                """h[m][:, o:o+cw] = relu(ps + b1[m]), alternating engines."""
                if m % 2 == 0:
                    nc.vector.tensor_scalar(
                        out=h_sb[m][:, o:o + cw], in0=ps_ap,
                        scalar1=b1_sb[:, m:m + 1], scalar2=0.0,
                        op0=add, op1=mx,
                    )
                else:
                    nc.scalar.activation(
                        out=h_sb[m][:, o:o + cw], in_=ps_ap,
                        func=relu, bias=b1_sb[:, m:m + 1],
                    )

            # ---- Phase A: k-outer streaming chunks ---------------------
            # The m-sweep splits (6, 2): the 6-group pass consumes each w1
            # k-tile in ~1.3us, matching the 2-ring arrival rate, and fits
            # the 7-buffer PSUM ring; the 2-group pass reruns the (by then
            # resident) k-tiles at full PE speed.
            for ci, (o, cw) in enumerate(a_chunks):
                for ms in (range(0, 6), range(6, 8)):
                    pss = {m: pspool.tile([P, 512], f32, name=f"psA_{ci}_{m}",
                                          tag="psring") for m in ms}
                    for k in range(KD):
                        for m in ms:
                            nc.tensor.matmul(
                                out=pss[m][:, :cw],
                                lhsT=w1_ap(k, m),
                                rhs=xt_sb[(k, ci)][:],
                                start=(k == 0),
                                stop=(k == KD - 1),
                            )
                    for m in ms:
                        relu_bias(m, pss[m][:, :cw], o, cw)
                # layer 2 for this chunk, k-outer over HID groups
                ps2a = pspool.tile([P, 512], f32, name=f"ps2A_{ci}", tag="psring")
                for k in range(KH):
                    nc.tensor.matmul(
                        out=ps2a[:, :cw],
                        lhsT=w2_sb[k],
                        rhs=h_sb[k][:, o:o + cw],
                        start=(k == 0),
                        stop=(k == KH - 1),
                    )
                ot = opool.tile([P, 512], f32, name=f"otA_{ci}", tag="ot")
                nc.vector.tensor_scalar_add(
                    out=ot[:C, :cw], in0=ps2a[:C, :cw], scalar1=b2_sb[:, 0:1],
                )
                nc.sync.dma_start(out=yt[:, o:o + cw], in_=ot[:C, :cw])

            # ---- Phase B: m-outer, tail folded into shared LDW ---------
            ob, wb = b_main
            ci_b = len(a_chunks)
            tw = b_tail[1] if b_tail else 0
            ps2b = ps2pool.tile([P, 512], f32, name="ps2b", tag="ps2b")
            # Tail layer-2: per-m single-shot matmuls accumulated on the DVE
            # into SBUF, so only one persistent PSUM bank (ps2b) is needed
            # and the ring keeps 7 buffers for phase A's 4+4 halves.
            yt_tail = (apool.tile([P, tw], f32, name="yt_tail", tag="yt_tail")
                       if b_tail else None)

            def l2_accum(m):
                nc.tensor.matmul(
                    out=ps2b[:, :wb],
                    lhsT=w2_sb[m],
                    rhs=h_sb[m][:, ob:ob + wb],
                    start=(m == 0),
                    stop=(m == KH - 1),
                )
                if b_tail:
                    pstl = pspool.tile([P, 512], f32, name=f"psl_{m}",
                                       tag="psring")
                    nc.tensor.matmul(
                        out=pstl[:, :tw],
                        lhsT=w2_sb[m],
                        rhs=h_sb[m][:, b_tail[0]:b_tail[0] + tw],
                        start=True, stop=True,
                    )
                    if m == 0:
                        nc.vector.tensor_scalar_add(
                            out=yt_tail[:C, :], in0=pstl[:C, :tw],
                            scalar1=b2_sb[:, 0:1],
                        )
                    else:
                        nc.vector.tensor_tensor(
                            out=yt_tail[:C, :], in0=yt_tail[:C, :],
                            in1=pstl[:C, :tw], op=add,
                        )

            for m in range(KH):
                ps1 = pspool.tile([P, 512], f32, name=f"psB_{m}", tag="psring")
                pst = (pspool.tile([P, 512], f32, name=f"psBt_{m}", tag="psring")
                       if b_tail else None)
                for k in range(KD):
                    nc.tensor.matmul(
                        out=ps1[:, :wb],
                        lhsT=w1_ap(k, m),
                        rhs=xt_sb[(k, ci_b)][:],
                        start=(k == 0),
                        stop=(k == KD - 1),
                    )
                    if b_tail:
                        nc.tensor.matmul(
                            out=pst[:, :tw],
                            lhsT=w1_ap(k, m),
                            rhs=xt_sb[(k, ci_b + 1)][:],
                            start=(k == 0),
                            stop=(k == KD - 1),
                        )
                    if k == 1 and m > 0:
                        # splice the previous m's layer-2 into this k-sweep
                        # so the PE never waits on relu(m-1)
                        l2_accum(m - 1)
                relu_bias(m, ps1[:, :wb], ob, wb)
                if b_tail:
                    relu_bias(m, pst[:, :tw], b_tail[0], tw)
            l2_accum(KH - 1)

            # final bias-adds + output, split across two DGE rings by
            # partition halves so the closing DMA is ~2x faster
            wtot = wb + tw
            otB = opool.tile([P, 544], f32, name="otB", tag="otB")
            nc.vector.tensor_scalar_add(
                out=otB[:C, 0:wb], in0=ps2b[:C, :wb], scalar1=b2_sb[:, 0:1],
            )
            if b_tail:
                nc.vector.tensor_copy(out=otB[:C, wb:wtot], in_=yt_tail[:C, :])
            h1 = C // 2
            nc.scalar.dma_start(out=yt[0:h1, ob:ob + wtot], in_=otB[0:h1, :wtot])
            nc.sync.dma_start(out=yt[h1:C, ob:ob + wtot], in_=otB[h1:C, :wtot])
    return nc


def _pad_cols(a, n):
    out = np.zeros((a.shape[0], n), dtype=a.dtype)
    out[:, :a.shape[1]] = a
    return out


def _route(task_id):
    """Group rows by task. Returns (row-index list per task, counts)."""
    task_id = np.asarray(task_id)
    order = np.argsort(task_id, kind="stable")
    counts = np.bincount(task_id.astype(np.int64), minlength=T)
    offs = np.zeros(T + 1, dtype=np.int64)
    np.cumsum(counts, out=offs[1:])
    rows = [order[offs[t]:offs[t + 1]] for t in range(T)]
    return rows, counts


def _run(inputs, trace=False):
    import ml_dtypes

    x = np.ascontiguousarray(np.asarray(inputs["x"], dtype=np.float32))
    task_id = np.asarray(inputs["task_id"])
    W1 = np.asarray(inputs["W1"], dtype=np.float32)
    b1 = np.asarray(inputs["b1"], dtype=np.float32)
    W2 = np.asarray(inputs["W2"], dtype=np.float32)
    b2 = np.asarray(inputs["b2"], dtype=np.float32)

    pad = int(os.environ.get("KMM_PAD", PAD_DEFAULT))
    n_warm = int(os.environ.get("KMM_WARM", "7"))
    rows, counts = _route(task_id)
    if counts.max() > pad:  # unexpected distribution: grow pad to fit
        pad = int(-(-int(counts.max()) // 16) * 16)

    io_np = ml_dtypes.bfloat16

    in_maps = []
    for t in range(T):
        xt = np.zeros((D, pad), dtype=io_np)
        xt[:, :counts[t]] = x[rows[t]].T
        in_maps.append({
            "xt": xt,
            "w1": np.ascontiguousarray(W1[t]).astype(io_np),
            "b1": np.ascontiguousarray(b1[t].reshape(KH, P).T.astype(np.float32)),
            "w2": _pad_cols(W2[t], P).astype(io_np),
            "b2": np.ascontiguousarray(b2[t][:, None].astype(np.float32)),
        })

    nc = build_program(pad, n_warm)
    nc.finalize()  # Bacc passes: legalize sync waits (<=1 per instruction)
    res = run_bass_kernel_spmd(
        nc, in_maps, core_ids=list(range(T)), trace=trace,
        trace_cores=list(range(T)) if trace else None,
        tmpdir=os.environ.get("KMM_TMPDIR"),
    )

    out = np.empty((task_id.shape[0], C), dtype=np.float32)
    for t in range(T):
        out[rows[t]] = res.results[t]["yt"][:, :counts[t]].T
    return out, res


def kernel(**inputs):
    out, _ = _run(inputs, trace=False)
    return out
